# revision 17
# baseline (speedup 1.0000x reference)
"""DLRM (embedding bag + dot interaction + MLPs) on 8 TRN2 NeuronCores.

Strategy: data-parallel over the batch (512 samples/core, full tables
replicated per core). Per core:
  - bottom MLP on PE (feature-major layouts)
  - embedding pooling: per (table, row-chunk) bucketed int16 dma_gather
    (chunk-local indices, window-grouped, padded), spread across the 4
    SWDGE queues (queue q -> Q7 core pair q) so descriptor generation
    for up to 4 gathers runs concurrently; gathered f32 rows are cast to
    bf16 on the Scalar engine; PE matmuls against bf16 one-hot selection
    matrices (built in one wide broadcast DVE op per bucket) accumulate
    pooled embeddings *transposed* ([d, bag]) directly in PSUM
  - dot interaction: per-4-sample Gram matmuls (bf16), DRAM round-trip to
    re-layout diagonal blocks into feature-major R tiles, PE transposes
  - top MLP consumes the full (symmetric) Gram via host-folded weights
All shapes/capacities are compile-time constants; inputs are rearranged
on the host (pure slicing/permutation + weight folding).
"""

import numpy as np

# ---------------- problem constants (hardcoded per contract) ----------------
T, N, D = 26, 100000, 64
B, L = 4096, 32
NCORES = 8
PNUM = 128

FULL_CFG = dict(
    T=T, N=N, D=D, L=L,
    BL=B // NCORES,          # 512 samples per core
    NCH=4,                   # chunks per table (chunk-local idx < 25000 -> int16)
    WCAP=1152,               # row capacity per (table, chunk, 128-bag window)
    LN_BOT=[13, 512, 256, 64],
    LN_TOP_H=[512, 256, 1],  # top hidden dims (input dim derived)
)


def _derived(cfg):
    d = dict(cfg)
    d["CHUNK"] = d["N"] // d["NCH"]
    d["NW"] = d["BL"] // PNUM            # windows (128-bag groups) per table
    d["BCAP"] = d["NW"] * d["WCAP"]      # gather capacity per bucket
    d["NBLK"] = d["BCAP"] // PNUM        # 128-row blocks per bucket
    d["NBLKW"] = d["WCAP"] // PNUM       # blocks per window
    d["NGB"] = d["BL"] * 32 // PNUM      # gram blocks (4 samples each)
    d["NRT"] = d["BL"] // PNUM           # sample tiles (R/RT)
    d["KG"] = 1024                       # padded gram feature count (32*32)
    assert d["CHUNK"] * d["NCH"] == d["N"] and d["CHUNK"] < 32768
    assert d["WCAP"] % PNUM == 0 and d["BL"] % PNUM == 0
    return d


# ---------------------------- device program ----------------------------

def build_program(cfg):
    import concourse.bass as bass
    import concourse.mybir as mybir
    from concourse import bacc, tile
    from contextlib import ExitStack

    c = _derived(cfg)
    Tn, Nn, Dn, Ln = c["T"], c["N"], c["D"], c["L"]
    BL, NCH, CHUNK = c["BL"], c["NCH"], c["CHUNK"]
    NW, WCAP, BCAP, NBLK, NBLKW = c["NW"], c["WCAP"], c["BCAP"], c["NBLK"], c["NBLKW"]
    NGB, NRT, KG = c["NGB"], c["NRT"], c["KG"]
    LNB = c["LN_BOT"]
    H1, H2, HOUT = c["LN_TOP_H"]
    assert HOUT == 1
    f32, i16 = mybir.dt.float32, mybir.dt.int16
    bf16 = mybir.dt.bfloat16
    RELU = mybir.ActivationFunctionType.Relu
    COPY = mybir.ActivationFunctionType.Copy
    IGCOL = BCAP // 16                   # gidx columns per bucket
    stage = cfg.get("stage", "full")     # debug: bot | pool | gram | full

    nc = bacc.Bacc("TRN2", target_bir_lowering=False, debug=False,
                   num_swdge_queues=4)

    # ---- DRAM I/O ----
    tab = nc.dram_tensor("tables", [Tn * Nn, Dn], f32, kind="ExternalInput")
    gidx = nc.dram_tensor("gidx", [PNUM, Tn * NCH * IGCOL], i16, kind="ExternalInput")
    rbid = nc.dram_tensor("rbid", [PNUM, Tn * NCH * NBLK], bf16, kind="ExternalInput")
    xT = nc.dram_tensor("xT", [LNB[0], BL], f32, kind="ExternalInput")
    iota_d = nc.dram_tensor("iota", [PNUM, PNUM], bf16, kind="ExternalInput")
    ident_d = nc.dram_tensor("ident", [PNUM, PNUM], f32, kind="ExternalInput")
    w0_d = nc.dram_tensor("w0", [LNB[0], LNB[1]], f32, kind="ExternalInput")
    w1_d = nc.dram_tensor("w1", [PNUM, (LNB[1] // PNUM) * LNB[2]], f32, kind="ExternalInput")
    w2_d = nc.dram_tensor("w2", [PNUM, (LNB[2] // PNUM) * LNB[3]], f32, kind="ExternalInput")
    b0_d = nc.dram_tensor("b0", [PNUM, LNB[1] // PNUM], f32, kind="ExternalInput")
    b1_d = nc.dram_tensor("b1", [PNUM, LNB[2] // PNUM], f32, kind="ExternalInput")
    b2_d = nc.dram_tensor("b2", [LNB[3], 1], f32, kind="ExternalInput")
    w1x_d = nc.dram_tensor("w1x", [Dn, H1], f32, kind="ExternalInput")
    w1z_d = nc.dram_tensor("w1z", [PNUM, (KG // PNUM) * H1], f32, kind="ExternalInput")
    tb1_d = nc.dram_tensor("tb1", [PNUM, H1 // PNUM], f32, kind="ExternalInput")
    w2t_d = nc.dram_tensor("w2t", [PNUM, (H1 // PNUM) * H2], f32, kind="ExternalInput")
    tb2_d = nc.dram_tensor("tb2", [PNUM, H2 // PNUM], f32, kind="ExternalInput")
    w3_d = nc.dram_tensor("w3", [PNUM, H2 // PNUM], f32, kind="ExternalInput")
    tb3_d = nc.dram_tensor("tb3", [1, 1], f32, kind="ExternalInput")
    out_d = nc.dram_tensor("out", [1, BL], f32, kind="ExternalOutput")
    dbg_d = (nc.dram_tensor("dbg", [Dn, BL * 32], bf16, kind="ExternalOutput")
             if stage in ("bot", "pool") else None)
    gdram = nc.dram_tensor("gdram", [PNUM, PNUM, PNUM], f32)  # [gram blk, p, f]
    assert NGB == PNUM

    NB1 = LNB[1] // PNUM   # 4 m-tiles of bottom h1
    NB2 = LNB[2] // PNUM   # 2 m-tiles of bottom h2
    NT1 = H1 // PNUM       # 4 m-tiles of top h1
    NT2 = H2 // PNUM       # 2 m-tiles of top h2
    NKG = KG // PNUM       # 8 gram K chunks

    with tile.TileContext(nc) as tc, ExitStack() as ctx:
        const = ctx.enter_context(tc.tile_pool(name="const", bufs=1))
        catp = ctx.enter_context(tc.tile_pool(name="catp", bufs=1))
        psA = ctx.enter_context(tc.tile_pool(name="psA", bufs=2, space="PSUM"))
        psMLP = ctx.enter_context(tc.tile_pool(name="psMLP", bufs=2, space="PSUM"))
        psSM = ctx.enter_context(tc.tile_pool(name="psSM", bufs=2, space="PSUM"))

        # ---- const loads ----
        iota_sb = const.tile([PNUM, PNUM], bf16, tag="iota")
        nc.sync.dma_start(iota_sb[:], iota_d[:])
        rbid_sb = const.tile([PNUM, Tn * NCH * NBLK], bf16, tag="rbid")
        nc.sync.dma_start(rbid_sb[:], rbid[:])

        # ---- catT: [64, BL*32] feature-major concatenated features (bf16) ----
        # col(sample b, cat-row i) = 32*(4*(b%128) + b//128) + i
        catT = catp.tile([Dn, BL * 32], bf16)
        # zero the pad cat rows (i >= 1+T within each sample's 32 cols)
        nc.vector.memset(
            catT[:].rearrange("d (s i) -> d s i", i=32)[:, :, 1 + Tn : 32], 0.0
        )

        # ---- bottom MLP (transient pool: space freed for pooling stage) ----
        mlp_ctx = tc.tile_pool(name="mlp", bufs=1)
        mlp = mlp_ctx.__enter__()
        xT_sb = mlp.tile([LNB[0], BL], f32, tag="xT")
        nc.sync.dma_start(xT_sb[:], xT[:])
        w0_sb = mlp.tile([LNB[0], LNB[1]], f32, tag="w0")
        nc.sync.dma_start(w0_sb[:], w0_d[:])
        w1_sb = mlp.tile([PNUM, (LNB[1] // PNUM) * LNB[2]], f32, tag="w1")
        nc.sync.dma_start(w1_sb[:], w1_d[:])
        w2_sb = mlp.tile([PNUM, (LNB[2] // PNUM) * LNB[3]], f32, tag="w2")
        nc.sync.dma_start(w2_sb[:], w2_d[:])
        b0_sb = mlp.tile([PNUM, NB1], f32, tag="b0")
        nc.sync.dma_start(b0_sb[:], b0_d[:])
        b1_sb = mlp.tile([PNUM, NB2], f32, tag="b1")
        nc.sync.dma_start(b1_sb[:], b1_d[:])
        b2_sb = mlp.tile([LNB[3], 1], f32, tag="b2")
        nc.sync.dma_start(b2_sb[:], b2_d[:])
        h1b = mlp.tile([PNUM, NB1 * BL], f32, tag="h1b")
        for m in range(NB1):
            ps = psMLP.tile([PNUM, BL], f32, tag="mlp")
            nc.tensor.matmul(ps[:], w0_sb[:, m * PNUM : (m + 1) * PNUM], xT_sb[:],
                             start=True, stop=True)
            nc.scalar.activation(h1b[:, m * BL : (m + 1) * BL], ps[:], RELU,
                                 bias=b0_sb[:, m : m + 1])
        h2b = mlp.tile([PNUM, NB2 * BL], f32, tag="h2b")
        for m in range(NB2):
            ps = psMLP.tile([PNUM, BL], f32, tag="mlp")
            for k in range(NB1):
                nc.tensor.matmul(
                    ps[:],
                    w1_sb[:, k * LNB[2] + m * PNUM : k * LNB[2] + (m + 1) * PNUM],
                    h1b[:, k * BL : (k + 1) * BL],
                    start=(k == 0), stop=(k == NB1 - 1))
            nc.scalar.activation(h2b[:, m * BL : (m + 1) * BL], ps[:], RELU,
                                 bias=b1_sb[:, m : m + 1])
        ps = psMLP.tile([PNUM, BL], f32, tag="mlp")
        for k in range(NB2):
            nc.tensor.matmul(ps[:LNB[3], :],
                             w2_sb[:, k * LNB[3] : (k + 1) * LNB[3]],
                             h2b[:, k * BL : (k + 1) * BL],
                             start=(k == 0), stop=(k == NB2 - 1))
        xeT = catp.tile([Dn, BL], f32, tag="xeT")
        nc.scalar.activation(xeT[:], ps[:LNB[3], :], RELU, bias=b2_sb[:])
        # scatter xe into catT col-0 slots: src col b=(hi*128+lo) -> dst 128*lo+32*hi
        nc.vector.tensor_copy(
            catT[:].rearrange("d (p g i) -> d p g i", p=PNUM, g=NW)[:, :, :, 0],
            xeT[:].rearrange("d (g p) -> d p g", g=NW),
        )
        mlp_ctx.__exit__(None, None, None)

        def finish_early():
            nc.sync.dma_start(dbg_d[:], catT[:])
            z = catp.tile([1, BL], f32, tag="zout")
            nc.vector.memset(z[:], 0.0)
            nc.sync.dma_start(out_d[:], z[:])

        if stage == "bot":
            finish_early()
        if stage != "bot":
            # ---- embedding pooling ----
            # gathers are chunk-major, one per (table, chunk) bucket, spread
            # across the 4 SWDGE queues (queue = chunk) so up to 4 descriptor
            # generations proceed in parallel on distinct Q7 core pairs.
            # Gathered f32 rows are cast to bf16 on the (otherwise idle)
            # Scalar engine; the bf16 one-hot for a whole bucket is built in
            # a single wide DVE op via broadcast access patterns.
            poolp_ctx = tc.tile_pool(name="poolp", bufs=12)
            gtp_ctx = tc.tile_pool(name="gtp", bufs=8)
            gbp_ctx = tc.tile_pool(name="gbp", bufs=8)
            stp_ctx = tc.tile_pool(name="stp", bufs=4)
            poolp = poolp_ctx.__enter__()
            gtp = gtp_ctx.__enter__()
            gbp = gbp_ctx.__enter__()
            stp = stp_ctx.__enter__()
            HBC = BCAP // 2        # idxs per half-gather
            HIG = IGCOL // 2       # gidx cols per half
            HNB = NBLK // 2        # blocks per half
            nreg = nc.gpsimd.to_reg(HBC)   # hoisted: one MOVE for all gathers
            for t in range(Tn):
                gbs = []
                for _ch in range(NCH):
                    gb = gbp.tile([PNUM, NBLK * Dn], bf16, tag="gb")
                    gbs.append(gb)
                sts = []
                # half-gathers issued h-outer/ch-inner so the queue sequence
                # is 0,1,2,3,0,1,2,3 and all 4 Q7 pairs stay busy
                for h in range(2):
                    for ch in range(NCH):
                        bk = t * NCH + ch
                        gix = poolp.tile([PNUM, HIG], i16, tag="gix")
                        nc.sync.dma_start(
                            gix[:],
                            gidx[:, bk * IGCOL + h * HIG : bk * IGCOL + (h + 1) * HIG])
                        gt = gtp.tile([PNUM, HNB * Dn], f32, tag="gt")
                        base = t * Nn + ch * CHUNK
                        nc.gpsimd.dma_gather(
                            gt[:].rearrange("p (b d) -> p b d", d=Dn),
                            tab[base : base + CHUNK, :],
                            gix[:],
                            HBC, nreg, Dn,
                            single_packet=False,
                            queue_num=ch,
                        )
                        nc.scalar.activation(
                            gbs[ch][:, h * HNB * Dn : (h + 1) * HNB * Dn],
                            gt[:], COPY)
                for ch in range(NCH):
                    bk = t * NCH + ch
                    # one-hot for the whole bucket in one DVE op:
                    # st[:, b*128+f] = (iota[:, f] == rbid[:, bk*NBLK+b])
                    st = stp.tile([PNUM, NBLK * PNUM], bf16, tag="st")
                    nc.vector.tensor_tensor(
                        out=st[:].rearrange("p (b f) -> p b f", b=NBLK),
                        in0=iota_sb[:].rearrange("p (o f) -> p o f", o=1)
                            .broadcast_to((PNUM, NBLK, PNUM)),
                        in1=rbid_sb[:, bk * NBLK : (bk + 1) * NBLK]
                            .rearrange("p (b o) -> p b o", o=1)
                            .broadcast_to((PNUM, NBLK, PNUM)),
                        op=mybir.AluOpType.is_equal,
                    )
                    sts.append(st)
                for w in range(NW):
                    pst = psA.tile([Dn, PNUM], f32, tag="acc")
                    for ch in range(NCH):
                        for bw in range(NBLKW):
                            b = w * NBLKW + bw
                            nc.tensor.matmul(
                                pst[:],
                                gbs[ch][:].rearrange("p (b d) -> p b d", d=Dn)[:, b, :],
                                sts[ch][:, b * PNUM : (b + 1) * PNUM],
                                start=(ch == 0 and bw == 0),
                                stop=(ch == NCH - 1 and bw == NBLKW - 1),
                            )
                    # pooled^T [d, p] -> catT cols 128*p + 32*w + (1+t)
                    nc.vector.tensor_copy(
                        catT[:].rearrange("d (p g i) -> d p g i", p=PNUM, g=NW)[:, :, w, 1 + t],
                        pst[:],
                    )

            stp_ctx.__exit__(None, None, None)
            gbp_ctx.__exit__(None, None, None)
            gtp_ctx.__exit__(None, None, None)
            poolp_ctx.__exit__(None, None, None)
            if stage == "pool":
                finish_early()

        if stage in ("gram", "full"):
            # ---- late consts (tail-only): keep the startup Sync queue clear ----
            ident_sb = const.tile([PNUM, PNUM], f32, tag="ident")
            nc.sync.dma_start(ident_sb[:], ident_d[:])
            w1x_sb = const.tile([Dn, H1], f32, tag="w1x")
            nc.sync.dma_start(w1x_sb[:], w1x_d[:])
            w1z_sb = const.tile([PNUM, NKG * H1], f32, tag="w1z")
            nc.sync.dma_start(w1z_sb[:], w1z_d[:])
            tb1_sb = const.tile([PNUM, NT1], f32, tag="tb1")
            nc.sync.dma_start(tb1_sb[:], tb1_d[:])
            w2t_sb = const.tile([PNUM, NT1 * H2], f32, tag="w2t")
            nc.sync.dma_start(w2t_sb[:], w2t_d[:])
            tb2_sb = const.tile([PNUM, NT2], f32, tag="tb2")
            nc.sync.dma_start(tb2_sb[:], tb2_d[:])
            w3_sb = const.tile([PNUM, NT2], f32, tag="w3")
            nc.sync.dma_start(w3_sb[:], w3_d[:])
            tb3_sb = const.tile([1, 1], f32, tag="tb3")
            nc.sync.dma_start(tb3_sb[:], tb3_d[:])
            # ---- gram blocks -> gdram ----
            gramp = ctx.enter_context(tc.tile_pool(name="gramp", bufs=2))
            for k4 in range(NGB // 4):
                stg = gramp.tile([PNUM, 4 * PNUM], f32, tag="stg")
                for j in range(4):
                    k = 4 * k4 + j
                    gps = psSM.tile([PNUM, PNUM], f32, tag="sm")
                    nc.tensor.matmul(gps[:], catT[:, k * PNUM : (k + 1) * PNUM],
                                     catT[:, k * PNUM : (k + 1) * PNUM],
                                     start=True, stop=True)
                    nc.vector.tensor_copy(stg[:, j * PNUM : (j + 1) * PNUM], gps[:])
                nc.sync.dma_start(
                    gdram[4 * k4 : 4 * k4 + 4, :, :].rearrange("k p f -> p k f"),
                    stg[:].rearrange("p (k f) -> p k f", k=4),
                )

            # ---- R tiles and RT (feature-major) ----
            topp = ctx.enter_context(tc.tile_pool(name="topp", bufs=2))
            RT = catp.tile([PNUM, NKG * BL], f32)  # row=feat%128 of chunk kc, col=sample
            for r in range(NRT):
                Rr = topp.tile([PNUM, KG], f32, tag="R")
                nc.sync.dma_start(
                    Rr[:].rearrange("p (i j) -> p i j", j=32),
                    gdram[:, 32 * r : 32 * r + 32, 32 * r : 32 * r + 32],
                )
                for kc in range(NKG):
                    tp = psSM.tile([PNUM, PNUM], f32, tag="sm")
                    nc.tensor.transpose(tp[:], Rr[:, kc * PNUM : (kc + 1) * PNUM],
                                        ident_sb[:])
                    nc.vector.tensor_copy(
                        RT[:, kc * BL + r * PNUM : kc * BL + (r + 1) * PNUM], tp[:])

            # ---- top MLP ----
            h1t = catp.tile([PNUM, NT1 * BL], f32)
            for m in range(NT1):
                ps = psMLP.tile([PNUM, BL], f32, tag="mlp")
                nc.tensor.matmul(ps[:], w1x_sb[:, m * PNUM : (m + 1) * PNUM], xeT[:],
                                 start=True, stop=False)
                for kc in range(NKG):
                    nc.tensor.matmul(
                        ps[:],
                        w1z_sb[:, kc * H1 + m * PNUM : kc * H1 + (m + 1) * PNUM],
                        RT[:, kc * BL : (kc + 1) * BL],
                        start=False, stop=(kc == NKG - 1))
                nc.scalar.activation(h1t[:, m * BL : (m + 1) * BL], ps[:], RELU,
                                     bias=tb1_sb[:, m : m + 1])
            h2t = catp.tile([PNUM, NT2 * BL], f32)
            for m in range(NT2):
                ps = psMLP.tile([PNUM, BL], f32, tag="mlp")
                for k in range(NT1):
                    nc.tensor.matmul(
                        ps[:],
                        w2t_sb[:, k * H2 + m * PNUM : k * H2 + (m + 1) * PNUM],
                        h1t[:, k * BL : (k + 1) * BL],
                        start=(k == 0), stop=(k == NT1 - 1))
                nc.scalar.activation(h2t[:, m * BL : (m + 1) * BL], ps[:], RELU,
                                     bias=tb2_sb[:, m : m + 1])
            ps = psMLP.tile([PNUM, BL], f32, tag="mlp")
            for k in range(NT2):
                nc.tensor.matmul(ps[:1, :], w3_sb[:, k : k + 1],
                                 h2t[:, k * BL : (k + 1) * BL],
                                 start=(k == 0), stop=(k == NT2 - 1))
            out_sb = catp.tile([1, BL], f32)
            nc.scalar.activation(out_sb[:], ps[:1, :], RELU, bias=tb3_sb[:])
            nc.sync.dma_start(out_d[:], out_sb[:])


    nc.compile()
    return nc


# ---------------------------- host-side prep ----------------------------

def make_core_inputs(inputs, core, cfg):
    import ml_dtypes
    bf16 = ml_dtypes.bfloat16
    c = _derived(cfg)
    Tn, Nn, Dn, Ln = c["T"], c["N"], c["D"], c["L"]
    BL, NCH, CHUNK = c["BL"], c["NCH"], c["CHUNK"]
    NW, WCAP, BCAP, NBLK = c["NW"], c["WCAP"], c["BCAP"], c["NBLK"]
    KG = c["KG"]
    LNB = c["LN_BOT"]
    H1, H2, _ = c["LN_TOP_H"]
    IGCOL = BCAP // 16

    sl = slice(core * BL, (core + 1) * BL)
    idx = np.asarray(inputs["indices"][:, sl, :]).astype(np.int64)  # [T, BL, L]

    gidx = np.zeros((Tn, NCH, BCAP), np.int16)
    rbid = np.full((Tn, NCH, BCAP), -1.0, np.float32)
    bag_of = np.repeat(np.arange(BL), Ln)
    for t in range(Tn):
        flat = idx[t].ravel()
        chunk = flat // CHUNK
        local = flat - chunk * CHUNK
        for ch in range(NCH):
            m = chunk == ch
            bags_m = bag_of[m]
            rows_m = local[m]
            for w in range(NW):
                wm = bags_m // PNUM == w
                n = int(wm.sum())
                if n > WCAP:
                    raise ValueError(f"window overflow {n} > {WCAP}")
                base = w * WCAP
                gidx[t, ch, base : base + n] = rows_m[wm]
                rbid[t, ch, base : base + n] = (bags_m[wm] % PNUM).astype(np.float32)
                if w % 2 == 1:
                    # trailing pad of each half-gather: -1 indices are trimmed
                    # by the gather ucode (descriptor generation skips them)
                    gidx[t, ch, base + n : base + WCAP] = -1

    # gather idx wrap-16 layout, replicated across 8 partition groups
    a = gidx.reshape(Tn * NCH, BCAP // 16, 16)
    a = np.swapaxes(a, 1, 2)                       # [buckets, 16, IGCOL]
    gidx_dram = np.tile(a, (1, 8, 1)).transpose(1, 0, 2).reshape(PNUM, Tn * NCH * IGCOL)
    gidx_dram = np.ascontiguousarray(gidx_dram)

    # rbid: position i -> [i%128, bucket*NBLK + i//128]
    r = rbid.reshape(Tn * NCH, NBLK, PNUM)
    rbid_dram = np.ascontiguousarray(
        r.transpose(2, 0, 1).reshape(PNUM, Tn * NCH * NBLK)).astype(bf16)

    def kmajor(w, kt):   # [K, M] -> [128, (K/128)*M] sbuf layout
        K, M = w.shape
        return np.ascontiguousarray(
            w.reshape(K // kt, kt, M).transpose(1, 0, 2).reshape(kt, (K // kt) * M))

    dense = np.asarray(inputs["dense_x"][sl]).astype(np.float32)
    w0 = np.asarray(inputs["bot_W0"], np.float32)      # [512, 13]
    w1 = np.asarray(inputs["bot_W1"], np.float32)      # [256, 512]
    w2 = np.asarray(inputs["bot_W2"], np.float32)      # [64, 256]
    b0 = np.asarray(inputs["bot_b0"], np.float32)
    b1 = np.asarray(inputs["bot_b1"], np.float32)
    b2 = np.asarray(inputs["bot_b2"], np.float32)
    tw1 = np.asarray(inputs["top_W0"], np.float32)     # [512, 64+T(T+1)/2]
    tw2 = np.asarray(inputs["top_W1"], np.float32)     # [256, 512]
    tw3 = np.asarray(inputs["top_W2"], np.float32)     # [1, 256]
    tb1 = np.asarray(inputs["top_b0"], np.float32)
    tb2 = np.asarray(inputs["top_b1"], np.float32)
    tb3 = np.asarray(inputs["top_b2"], np.float32)

    # fold tril Zflat columns into the full (padded 32x32) gram layout
    ncat = Tn + 1
    w1z = np.zeros((KG, H1), np.float32)
    li, lj = np.tril_indices(ncat, -1)
    for k in range(len(li)):
        a_, b_ = int(li[k]), int(lj[k])
        colw = tw1[:, Dn + k] * 0.5
        w1z[32 * a_ + b_] += colw
        w1z[32 * b_ + a_] += colw

    return {
        "tables": np.asarray(inputs["tables"], np.float32).reshape(Tn * Nn, Dn),
        "gidx": gidx_dram,
        "rbid": rbid_dram,
        "xT": np.ascontiguousarray(dense.T),
        "iota": np.ascontiguousarray(
            np.tile(np.arange(PNUM, dtype=np.float32), (PNUM, 1))).astype(bf16),
        "ident": np.eye(PNUM, dtype=np.float32),
        "w0": np.ascontiguousarray(w0.T),                       # [13, 512]
        "w1": kmajor(w1.T, PNUM),                               # W1T [512,256]
        "w2": kmajor(w2.T, PNUM),                               # W2T [256,64]
        "b0": np.ascontiguousarray(b0.reshape(-1, PNUM).T),
        "b1": np.ascontiguousarray(b1.reshape(-1, PNUM).T),
        "b2": np.ascontiguousarray(b2.reshape(-1, 1)),
        "w1x": np.ascontiguousarray(tw1[:, :Dn].T),             # [64, 512]
        "w1z": kmajor(w1z, PNUM),
        "tb1": np.ascontiguousarray(tb1.reshape(-1, PNUM).T),
        "w2t": kmajor(tw2.T, PNUM),
        "tb2": np.ascontiguousarray(tb2.reshape(-1, PNUM).T),
        "w3": kmajor(tw3.T, PNUM),
        "tb3": np.ascontiguousarray(tb3.reshape(1, 1)),
    }


# ---------------------------- entry point ----------------------------

_CACHE = {}


def _get_program():
    if "nc" not in _CACHE:
        _CACHE["nc"] = build_program(FULL_CFG)
    return _CACHE["nc"]


def run_cores(inputs, trace=False, trace_kwargs=None):
    from concourse.bass_utils import run_bass_kernel_spmd

    nc = _get_program()
    in_maps = [make_core_inputs(inputs, c, FULL_CFG) for c in range(NCORES)]
    res = run_bass_kernel_spmd(
        nc, in_maps, list(range(NCORES)), trace=trace, **(trace_kwargs or {}))
    out = np.concatenate([res.results[c]["out"][0] for c in range(NCORES)])
    return out.reshape(B, 1).astype(np.float32), res


def kernel(**inputs):
    out, _ = run_cores(inputs)
    return out
